# revision 13
# baseline (speedup 1.0000x reference)
"""Trainium2 Bass kernel for nn_ExpertCompoundTracker (histogram_binning).

Math: with h_t the [E]-dim multiplicity vector of token t's TOP_K expert
indices (sum of K one-hots), the reference outputs are

    counts        = sum_t h_t                  (load histogram * N)
    coact_delta   = sum_t h_t h_t^T - diag(counts)
    new_load_ema  = ema * 0.99 + (counts/N) * 0.01
    new_coact     = coact_in + coact_delta

The device computes Q_core = sum_t h_t h_t^T per core (data-parallel over
tokens) as a long PSUM-accumulated chain of [128,64]x[128,64] matmuls.
Everything is exact integer arithmetic in bf16/f32 (values tiny).
counts falls out for free on the host: row-sums of Q are 4*counts because
every token contributes exactly K=4 slots.

Device pipeline per core (262144 tokens):
  - indices are shipped from host as bf16 (ints 0..63 exact in bf16),
    laid out [128 partitions, K=4 slot-planes, 2048 tokens].
  - per block of CB tokens-per-partition: 4 DVE is_equal compares of a
    constant iota-replicate tile [128, E*CB] against the slot value
    broadcast (free-dim step-0 AP), then 3 adds -> H block [128, (E, CB)].
    All APs are inner-step-1 bf16 so DVE runs in 2x mode.
  - per token column c: matmul(lhsT=H[:, :, c], rhs=H[:, :, c]) accumulating
    into one PSUM [64, 64] f32 tile (exact: max 262144*16 < 2^24).
  - PSUM -> SBUF -> DRAM out [64, 64] f32.

Host: sums the 8 per-core Q's in f64 and applies the tiny epilogue.
"""

import numpy as np
import ml_dtypes
from contextlib import ExitStack

import concourse.bass as bass
import concourse.mybir as mybir
import concourse.tile as tile
from concourse.bass_utils import run_bass_kernel_spmd

from concourse.vector_clock import ScopedClock


def _split_drain_and_barrier(self, tick_clock, wait_clock):
    """Replacement for TileContext._drain_and_barrier emitting one sync wait
    per drain instruction — this walrus build rejects instructions carrying
    more than one wait condition ("Too many sync wait commands")."""
    nc = self.nc
    drain_inst = nc.sync.drain()
    wait_clock.add_sem_waits(
        drain_inst.ins, ScopedClock({None: tick_clock.global_clock}))
    si = drain_inst.ins.sync_info
    if si is not None and si.on_wait and len(si.on_wait) > 1:
        extra = list(si.on_wait[1:])
        del si.on_wait[1:]
        for w in extra:
            d2 = nc.sync.drain()
            si2 = d2.ins.sync_info
            if si2 is None:
                d2.ins.sync_info = mybir.SyncInfo(on_wait=[w], on_update=[])
            else:
                si2.on_wait.append(w)
    nc.all_engine_barrier()
    assert self.sems is not None
    popped = nc._tile_sem_poison_stack.pop()
    assert popped is self._sem_poison
    nc.clear_and_free_semaphores(list(self.sems.allocated().values()))
    nc.all_engine_barrier()


tile.TileContext._drain_and_barrier = _split_drain_and_barrier

N_CORES = 8
N_TOKENS = 2097152
K = 4
E = 64
EMA_DECAY = 0.99

P = 128                      # SBUF partitions
TPC = N_TOKENS // N_CORES    # tokens per core = 262144
TPP = TPC // P               # tokens per partition = 2048
CB = 64                      # tokens-per-partition per compute block
NBLK = TPP // CB             # blocks
BF16 = mybir.dt.bfloat16
F32 = mybir.dt.float32

_CACHE = {}


def _build_bass():
    nc = bass.Bass("TRN2", target_bir_lowering=False, debug=False,
                   num_devices=N_CORES)

    # x and the iota-replicate constant ride one DRAM tensor / one DMA so
    # downstream DVE ops carry a single semaphore wait (TT allows only one).
    x_in = nc.dram_tensor("x", [P, K * TPP + E * CB], BF16,
                          kind="ExternalInput")
    q_out = nc.dram_tensor("q", [E, E], F32, kind="ExternalOutput")

    # Input/output staging buffers live outside the tile pools and both DMAs
    # run as raw prologue/epilogue blocks around the TileContext. The tile
    # region then has no DMA semaphores at all, keeping the kernel-tail
    # drain's wait list at {PE, DVE} (walrus caps sync waits per instruction).
    qs = nc.alloc_sbuf_tensor("qs", [E, E], F32)
    xt_raw = nc.alloc_sbuf_tensor("xt", [P, K * TPP + E * CB], BF16)

    with nc.semaphore("in_sem") as isem, nc.Block() as block:
        @block.sync
        def _(sync):
            sync.dma_start(out=xt_raw.ap(), in_=x_in.ap()).then_inc(isem, 16)
            sync.wait_ge(isem, 16)

    with tile.TileContext(nc) as tc, ExitStack() as ctx:
        hpool = ctx.enter_context(tc.tile_pool(name="h", bufs=2))
        tpool = ctx.enter_context(tc.tile_pool(name="tmp", bufs=3))
        ppool = ctx.enter_context(tc.tile_pool(name="psum", bufs=1, space="PSUM"))

        xt = xt_raw.ap()
        io = xt[:, K * TPP: K * TPP + E * CB]

        q = ppool.tile([E, E], F32)

        n_mm = NBLK * CB
        mm = 0
        for blk in range(NBLK):
            # broadcast APs of each slot's CB token values along the E axis
            def xb(k):
                sl = xt[:, k * TPP + blk * CB: k * TPP + (blk + 1) * CB]
                return sl.unsqueeze(1).broadcast_to([P, E, CB])

            io3 = io.rearrange("p (e c) -> p e c", e=E)

            h = hpool.tile([P, E * CB], BF16, tag="h")

            eq = mybir.AluOpType.is_equal
            # serial in-place accumulation: each DVE op carries at most one
            # cross-engine semaphore wait (TT instructions allow only one).
            nc.vector.tensor_tensor(
                h[:].rearrange("p (e c) -> p e c", e=E), io3, xb(0), op=eq)
            for k in range(1, K):
                t = tpool.tile([P, E * CB], BF16, tag="t")
                nc.vector.tensor_tensor(
                    t[:].rearrange("p (e c) -> p e c", e=E), io3, xb(k), op=eq)
                nc.vector.tensor_add(h[:], h[:], t[:])

            h3 = h[:].rearrange("p (e c) -> p c e", e=E)
            for c in range(CB):
                nc.tensor.matmul(
                    q[:], h3[:, c, :], h3[:, c, :],
                    start=(mm == 0), stop=(mm == n_mm - 1))
                mm += 1

        nc.vector.tensor_copy(qs.ap(), q[:])

    # Past the tile drain every engine is idle and qs is final; a raw DMA
    # ships it out with its own semaphore.
    with nc.semaphore("out_sem") as osem, nc.Block() as block:
        @block.sync
        def _(sync):
            sync.dma_start(out=q_out.ap(), in_=qs.ap()).then_inc(osem, 16)
            sync.wait_ge(osem, 16)

    return nc


def _marshal_inputs(expert_indices):
    idx = np.asarray(expert_indices)
    iota = np.tile(np.repeat(np.arange(E, dtype=np.float32), CB), (P, 1))
    iota = iota.astype(ml_dtypes.bfloat16)                      # [P, E*CB]
    xs = []
    for c in range(N_CORES):
        sl = idx[c * TPC:(c + 1) * TPC].astype(np.float32)      # [TPC, K]
        sl = sl.reshape(P, TPP, K).transpose(0, 2, 1)           # [P, K, TPP]
        x = np.empty((P, K * TPP + E * CB), dtype=ml_dtypes.bfloat16)
        x[:, :K * TPP] = sl.reshape(P, K * TPP).astype(ml_dtypes.bfloat16)
        x[:, K * TPP:] = iota
        xs.append(x)
    return xs


def kernel(expert_indices, expert_weights, expert_load_ema,
           expert_pair_coactivation):
    if "nc" not in _CACHE:
        _CACHE["nc"] = _build_bass()
    nc = _CACHE["nc"]

    xs = _marshal_inputs(expert_indices)
    in_maps = [{"x": xs[c]} for c in range(N_CORES)]
    res = run_bass_kernel_spmd(nc, in_maps, core_ids=list(range(N_CORES)))

    Q = np.zeros((E, E), dtype=np.float64)
    for c in range(N_CORES):
        Q += np.asarray(res.results[c]["q"], dtype=np.float64)

    counts = Q.sum(axis=1) / K
    coact_delta = Q - np.diag(counts)
    load = counts / N_TOKENS

    ema = np.asarray(expert_load_ema, dtype=np.float64)
    coact_in = np.asarray(expert_pair_coactivation, dtype=np.float64)
    new_ema = (ema * EMA_DECAY + load * (1.0 - EMA_DECAY)).astype(np.float32)
    new_coact = (coact_in + coact_delta).astype(np.float32)
    return new_ema, new_coact
